# revision 35
# baseline (speedup 1.0000x reference)
"""Deformable PSROI pooling (group_size=1, num_classes=1) on 8 trn2 NeuronCores.

Strategy ("block sweep", v2):
  out[n, c, ph, pw] = sum_{y,x} KY[bin, y] * KX[bin, x] * data[b, c, y, x]
with KX/KY per-bin bilinear hat profiles (masks and 1/count folded in).
Support is tiny (<=5 rows x <=5 cols), so the contraction is blocked into
4-row x 32-col map blocks packed into the PE partition dim:
  map4[p = (y%4)*32 + (x%32), (yblk a, xblk b, c)]
One K=128 matmul per (gen, block) contracts a whole 4x32 block for all bins
of that block:   psum[c, cols] += map4[:, a, b, :].T @ W[:, cols]
where W[p, col] = KX[bin, 32b + p%32] * KY[bin, 4a + p//32]  (host-built).

Bins are grouped into (ylo-window, home-xblk) units; units are cap-equalized
across cores (column layout shared by the single compiled program) and packed
into <=512-column PSUM banks. Per unit: a main matmul (full unit range), a
straddle-right matmul over the straddler sub-segment (block xb+1), and spill
matmuls into block a+1 over descending-yhi-sorted prefixes (support <= 5 rows
=> a bin touches at most 2 y-windows / 2 xblks). W values are support-driven:
columns outside a bin's support are zero, which makes cap padding and foreign
columns in shared envelopes harmless.

Sharding: RoI bins are split by (batch, ylo-quantile) into 8 shards; the
compiled program is shared, per-core variation lives in tensor contents.
All streams are bf16 (tolerance 2e-2; bf16 error ~5e-3 on hardware).
"""
import sys
import time

import numpy as np

sys.path.insert(0, "/opt/trn_rl_repo")

SPATIAL_SCALE = np.float32(0.0625)
POOLED = 7
SAMPLES = 4
TRANS_STD = np.float32(0.1)
B, C, H, W = 2, 128, 128, 128
NCORES = 8
GEN_COLS = 512
YWIN = 4
XWIN = 32
NXB = W // XWIN

f32 = np.float32
YSENT = 10 ** 6


def _bf16(a):
    import ml_dtypes
    return a.astype(ml_dtypes.bfloat16)


# ----------------------------------------------------------------------------
# host planning
# ----------------------------------------------------------------------------

def _bin_params(rois, offset):
    """Exact float32 emulation of the reference coordinate math.

    Returns per-bin (N*49) arrays: batch, dense hat profiles kx/ky [nb, 128]
    (ky has 1/count folded in), y-support [ylo, yhi], active mask.
    """
    N = rois.shape[0]
    P, S = POOLED, SAMPLES
    rois = rois.astype(f32)
    offset = offset.astype(f32)

    batch_ind = rois[:, 0].astype(np.int32)
    roi_sw = np.round(rois[:, 1]) * SPATIAL_SCALE - f32(0.5)
    roi_sh = np.round(rois[:, 2]) * SPATIAL_SCALE - f32(0.5)
    roi_ew = np.round(rois[:, 3] + f32(1.0)) * SPATIAL_SCALE - f32(0.5)
    roi_eh = np.round(rois[:, 4] + f32(1.0)) * SPATIAL_SCALE - f32(0.5)
    roi_w = np.maximum(roi_ew - roi_sw, f32(0.1))
    roi_h = np.maximum(roi_eh - roi_sh, f32(0.1))
    bin_w = roi_w / f32(P)
    bin_h = roi_h / f32(P)
    sub_w = bin_w / f32(S)
    sub_h = bin_h / f32(S)

    pidx = np.arange(P, dtype=f32)
    trans_x = offset[:, 0] * TRANS_STD
    trans_y = offset[:, 1] * TRANS_STD
    pw = pidx[None, None, :]
    ph = pidx[None, :, None]
    wstart = pw * bin_w[:, None, None] + roi_sw[:, None, None] + trans_x * roi_w[:, None, None]
    hstart = ph * bin_h[:, None, None] + roi_sh[:, None, None] + trans_y * roi_h[:, None, None]

    sidx = np.arange(S, dtype=f32)
    w_s = wstart[..., None] + sidx * sub_w[:, None, None, None]
    h_s = hstart[..., None] + sidx * sub_h[:, None, None, None]
    mask_w = (w_s >= f32(-0.5)) & (w_s <= f32(W) - f32(0.5))
    mask_h = (h_s >= f32(-0.5)) & (h_s <= f32(H) - f32(0.5))
    wc = np.clip(w_s, f32(0.0), f32(W - 1))
    hc = np.clip(h_s, f32(0.0), f32(H - 1))

    cnt = (mask_h.sum(-1) * mask_w.sum(-1)).astype(f32)
    inv = np.where(cnt > 0, f32(1.0) / np.maximum(cnt, f32(1.0)), f32(0.0))

    nb = N * P * P
    wc = wc.reshape(nb, S)
    hc = hc.reshape(nb, S)
    mask_w = mask_w.reshape(nb, S)
    mask_h = mask_h.reshape(nb, S)
    inv = inv.reshape(nb)

    xg = np.arange(W, dtype=np.float64)
    kx = np.zeros((nb, W), np.float64)
    ky = np.zeros((nb, H), np.float64)
    for s in range(S):
        kx += mask_w[:, s, None] * np.maximum(0.0, 1.0 - np.abs(wc[:, s, None].astype(np.float64) - xg))
        ky += mask_h[:, s, None] * np.maximum(0.0, 1.0 - np.abs(hc[:, s, None].astype(np.float64) - xg))
    ky *= inv[:, None]
    kx = kx.astype(f32)
    ky = ky.astype(f32)

    ky_nz = ky != 0
    kx_nz = kx != 0
    act = ky_nz.any(axis=1) & kx_nz.any(axis=1)
    ylo = np.where(act, ky_nz.argmax(axis=1), YSENT).astype(np.int64)
    yhi = np.where(act, H - 1 - ky_nz[:, ::-1].argmax(axis=1), -YSENT).astype(np.int64)
    xlo = np.where(act, kx_nz.argmax(axis=1), 0).astype(np.int64)
    xhi = np.where(act, W - 1 - kx_nz[:, ::-1].argmax(axis=1), 0).astype(np.int64)

    batch = np.repeat(batch_ind, P * P)
    return batch, kx, ky, ylo, yhi, xlo, xhi, act


def _plan(rois, offset):
    batch, kx, ky, ylo, yhi, xlo, xhi, act = _bin_params(rois, offset)

    # cell = home xblk; straddlers sort as a suffix inside their home cell
    home = np.clip(xlo // XWIN, 0, NXB - 1)
    strad = (xhi // XWIN) > home
    cell = home
    ncells = NXB

    # shard per batch into 4 ylo-quantile shards
    shards = []
    for b in range(B):
        ids = np.where((batch == b) & act)[0]
        ids = ids[np.lexsort((yhi[ids], ylo[ids]))]
        q = NCORES // B
        shards.extend(ids[int(len(ids) * i / q):int(len(ids) * (i + 1) / q)]
                      for i in range(q))
    assert len(shards) == NCORES

    # per-core 4-aligned row start
    row_start = np.zeros(NCORES, np.int64)
    for ci, ids in enumerate(shards):
        row_start[ci] = (int(ylo[ids].min()) // YWIN) * YWIN if len(ids) else 0
    nyb = 0
    for ci, ids in enumerate(shards):
        if len(ids):
            nyb = max(nyb, (int(yhi[ids].max()) - int(row_start[ci])) // YWIN + 1)
    nyb += 1  # room for the spill block of the last window
    ngens_max = nyb  # window index range

    # per (core, gen): bins with local ylo window == g, ordered by
    # (cell, straddle-flag, -yhi) -- straddlers are a sub-capped segment per
    # cell; within each segment spillers (yhi >= window end) are an exact
    # PREFIX (descending yhi), so spill envelopes anchor at the segment base
    core_gen = {}
    for ci, ids in enumerate(shards):
        g_of = (ylo[ids] - row_start[ci]) // YWIN
        for g in range(ngens_max):
            sub = ids[g_of == g]
            sub = sub[np.lexsort((-yhi[sub], strad[sub], cell[sub]))]
            core_gen[(ci, g)] = sub

    spans_ok = True
    for ids in shards:
        if len(ids):
            spans_ok &= bool((yhi[ids] - ylo[ids]).max() <= 2 * YWIN - 1)
            spans_ok &= bool((xhi[ids] - xlo[ids]).max() <= XWIN - 1)
    assert spans_ok, "bin support exceeds block-spill capacity"

    # units = (window g, cell e) groups; greedily packed into <=512-col banks.
    # mms entry: (a, b, cl, ch, wc); W values are support-driven (auto-zero
    # outside support), so one fill rule covers main/straddle-right/spill.
    units = []
    for g in range(ngens_max):
        if max(len(core_gen[(ci, g)]) for ci in range(NCORES)) == 0:
            continue
        for e in range(ncells):
            lp, ls = [], []
            for ci in range(NCORES):
                sub = core_gen[(ci, g)]
                mine = sub[cell[sub] == e]
                lp.append(mine[~strad[mine]])
                ls.append(mine[strad[mine]])
            capp = max(len(l_) for l_ in lp)
            caps_ = max(len(l_) for l_ in ls)
            if capp + caps_:
                units.append((g, e, capp, caps_, lp, ls))

    gens = []
    colbin = {}         # (ci, bank_idx) -> bin id per column (-1 hole)
    out_off = 0
    wm_cur = 0
    u0 = 0
    while u0 < len(units):
        u1 = u0 + 1
        cw = units[u0][2] + units[u0][3]
        while u1 < len(units) and cw + units[u1][2] + units[u1][3] <= GEN_COLS:
            cw += units[u1][2] + units[u1][3]
            u1 += 1
        capw = cw
        wm_lo = wm_cur
        mms = []
        for ci in range(NCORES):
            cb = -np.ones(capw, np.int64)
            b_ = 0
            for (g, e, capp, caps_, lp, ls) in units[u0:u1]:
                cb[b_:b_ + len(lp[ci])] = lp[ci]
                cb[b_ + capp:b_ + capp + len(ls[ci])] = ls[ci]
                b_ += capp + caps_
            colbin[(ci, len(gens))] = cb
        base = 0
        for (g, e, capp, caps_, lp, ls) in units[u0:u1]:
            T = YWIN * (g + 1)
            # spill prefix lengths (descending-yhi sort => exact prefix)
            nsp, nss = 0, 0
            for ci in range(NCORES):
                if len(lp[ci]):
                    nsp = max(nsp, int(np.count_nonzero(
                        yhi[lp[ci]] - row_start[ci] >= T)))
                if len(ls[ci]):
                    nss = max(nss, int(np.count_nonzero(
                        yhi[ls[ci]] - row_start[ci] >= T)))
            # main: block (g, e) over the whole unit range
            mms.append(dict(a=g, b=e, cl=base, ch=base + capp + caps_,
                            wc=wm_cur))
            wm_cur += capp + caps_
            # straddle-right: block (g, e+1) over the straddler sub-segment
            if caps_ and e + 1 < NXB:
                mms.append(dict(a=g, b=e + 1, cl=base + capp,
                                ch=base + capp + caps_, wc=wm_cur))
                wm_cur += caps_
            # spill: block (g+1, e) over the spill prefix(es)
            if nsp:
                mms.append(dict(a=g + 1, b=e, cl=base, ch=base + nsp,
                                wc=wm_cur))
                wm_cur += nsp
            if nss:
                mms.append(dict(a=g + 1, b=e, cl=base + capp,
                                ch=base + capp + nss, wc=wm_cur))
                wm_cur += nss
                if e + 1 < NXB:
                    mms.append(dict(a=g + 1, b=e + 1, cl=base + capp,
                                    ch=base + capp + nss, wc=wm_cur))
                    wm_cur += nss
            base += capp + caps_
        gens.append(dict(g=units[u0][0], capw=capw, out_off=out_off, mms=mms,
                         wm_lo=wm_lo, wm_hi=wm_cur))
        out_off += -(-capw // 8) * 8  # pad to 8 cols (16B bf16 alignment)
        u0 = u1
    nslots = out_off
    WCm = max(wm_cur, 8)

    # trim: keep only map blocks some matmul references, in (a, b) order
    used = sorted({(m["a"], m["b"]) for gd in gens for m in gd["mms"]})
    blkid = {ab: i for i, ab in enumerate(used)}
    for gd in gens:
        for m in gd["mms"]:
            m["blk"] = blkid[(m["a"], m["b"])]
    nblk = len(used)

    # wm chunk boundaries (per ~2 gens) for streaming
    wm_chunks = []
    for i in range(0, len(gens), 2):
        j = min(i + 2, len(gens))
        wm_chunks.append((i, j, gens[i]["wm_lo"], gens[j - 1]["wm_hi"]))

    # hashable meta for the device program
    meta_gens = []
    for gd in gens:
        mt = tuple((m["blk"], m["cl"], m["ch"], m["wc"]) for m in gd["mms"])
        meta_gens.append((gd["g"], gd["capw"], gd["out_off"], mt))
    meta = dict(nyb=int(nyb), nblk=int(nblk), nslots=int(nslots), WCm=int(WCm),
                gens=tuple(meta_gens),
                wm_chunks=tuple(wm_chunks))
    return dict(meta=meta, gens=gens, row_start=row_start, colbin=colbin,
                kx=kx, ky=ky, act=act, batch=batch, used_blocks=tuple(used))


def _build_inputs(plan, data):
    meta = plan["meta"]
    nyb, WCm = meta["nyb"], meta["WCm"]
    kx, ky = plan["kx"], plan["ky"]
    gens, colbin, row_start = plan["gens"], plan["colbin"], plan["row_start"]

    used = plan["used_blocks"]
    in_maps = []
    for ci in range(NCORES):
        b = ci // (NCORES // B)
        rs = int(row_start[ci])
        # map4[p=(dy*32+dx), (block, c)] for referenced (a, xb) blocks only
        D = data[b]                           # [C, H, W] f32
        rows = np.zeros((C, nyb * YWIN, W), f32)
        r1 = min(H, rs + nyb * YWIN)
        rows[:, :r1 - rs, :] = D[:, rs:r1, :]
        m4f = rows.reshape(C, nyb, YWIN, NXB, XWIN)
        m4f = m4f.transpose(2, 4, 1, 3, 0)    # dy, dx, a, xb, c
        m4 = np.empty((YWIN, XWIN, len(used), C), f32)
        for i, (a, b_) in enumerate(used):
            m4[:, :, i, :] = m4f[:, :, a, b_, :]
        m4 = m4.reshape(128, len(used) * C)

        wm = np.zeros((128, WCm), f32)
        for gi, gd in enumerate(gens):
            cb = colbin[(ci, gi)]
            for m in gd["mms"]:
                a, b_, cl, ch, wc = m["a"], m["b"], m["cl"], m["ch"], m["wc"]
                q = cb[cl:ch]
                sel = q >= 0
                if not sel.any():
                    continue
                qs = q[sel]
                jsel = np.nonzero(sel)[0]
                xs = np.arange(XWIN * b_, XWIN * (b_ + 1))
                kxv = kx[qs][:, xs]                      # [n, 32]
                ys = rs + YWIN * a + np.arange(YWIN)
                valid = ys < H
                kyv = np.zeros((len(qs), YWIN), f32)
                kyv[:, valid] = ky[qs][:, ys[valid]]     # [n, 4]
                vals = kyv[:, :, None] * kxv[:, None, :]
                wm[:, wc + jsel] = vals.reshape(len(qs), 128).T
        in_maps.append({"mp": _bf16(m4), "wm": _bf16(wm)})
    return in_maps


# ----------------------------------------------------------------------------
# host emulation (for plan debugging; mirrors the device program exactly)
# ----------------------------------------------------------------------------

def _emulate(plan, in_maps):
    meta = plan["meta"]
    nslots = meta["nslots"]
    outs = []
    for ci in range(NCORES):
        m4 = in_maps[ci]["mp"].astype(f32)
        wm = in_maps[ci]["wm"].astype(f32)
        o = np.zeros((128, nslots), f32)
        for (g, capw, out_off, mms) in meta["gens"]:
            ps = np.zeros((128, GEN_COLS), f32)
            for (bi, cl, ch, wc) in mms:
                blk = m4[:, bi * C:(bi + 1) * C]
                ps[:, cl:ch] += blk.T @ wm[:, wc:wc + ch - cl]
            o[:, out_off:out_off + capw] = ps[:, :capw]
        outs.append(o)
    return outs


def _gather(plan, outs):
    meta = plan["meta"]
    N = plan["batch"].shape[0] // (POOLED * POOLED)
    flat = np.zeros((N * POOLED * POOLED, C), f32)
    for ci in range(NCORES):
        o = outs[ci]
        for gi, (g, capw, out_off, mms) in enumerate(meta["gens"]):
            cb = plan["colbin"][(ci, gi)]
            sel = cb >= 0
            if sel.any():
                flat[cb[sel]] = o[:, out_off:out_off + capw][:, sel].T
    flat[~plan["act"]] = 0.0
    out = flat.reshape(N, POOLED, POOLED, C).transpose(0, 3, 1, 2)
    return np.ascontiguousarray(out)


# ----------------------------------------------------------------------------
# device program
# ----------------------------------------------------------------------------

def _split_drains(nc, mybir, bass_rust):
    for f_ in nc.m.functions:
        for blk in f_.blocks:
            newlist = []
            for ins in blk.instructions:
                wts = list(ins.sync_info.on_wait) if ins.sync_info else []
                if len(wts) > 1 and type(ins).__name__ == "InstDrain":
                    for j, wx in enumerate(wts[1:]):
                        nop = mybir.InstNoOp(name=f"splitw_{id(ins)}_{j}", ins=[], outs=[])
                        nop.engine = ins.engine
                        nop.sync_info = bass_rust.SyncInfo(on_wait=[wx], on_update=[])
                        newlist.append(nop)
                    ins.sync_info.on_wait = wts[:1]
                newlist.append(ins)
            blk.instructions = newlist


def _build_program(meta, rep=1):
    import concourse.bacc as bacc
    import concourse.mybir as mybir
    import bass_rust
    from concourse.tile import TileContext

    nblk, nslots = meta["nblk"], meta["nslots"]
    WCm = meta["WCm"]
    gens, wm_chunks = meta["gens"], meta["wm_chunks"]
    dt = mybir.dt.bfloat16

    nc = bacc.Bacc()
    mp = nc.declare_dram_parameter("mp", [128, nblk * C], dt, isOutput=False)
    wm = nc.declare_dram_parameter("wm", [128, WCm], dt, isOutput=False)
    o = nc.declare_dram_parameter("o", [128, nslots], dt, isOutput=True)

    wm_max = max(hi - lo for (_, _, lo, hi) in wm_chunks) if wm_chunks else 8

    with TileContext(nc) as tc:
        with (
            tc.tile_pool(name="const", bufs=1) as constp,
            tc.tile_pool(name="wp", bufs=3) as wpool,
            tc.tile_pool(name="ps", bufs=2, space="PSUM") as pspool,
        ):
            for _rep in range(rep):
                map_t = constp.tile([128, nblk * C], dt, tag="map")
                stage = constp.tile([128, nslots], dt, tag="stage")
                # map: 8 slices on SP ring, at block boundaries
                nload = min(8, nblk)
                bounds = [int(nblk * i / nload) for i in range(nload + 1)]
                for i in range(nload):
                    a0, a1 = bounds[i], bounds[i + 1]
                    if a1 > a0:
                        nc.sync.dma_start(
                            out=map_t[:, a0 * C:a1 * C],
                            in_=mp[:, a0 * C:a1 * C])

                wm_t = {}
                for (g0, g1, lo, hi) in wm_chunks:
                    if hi > lo:
                        t = wpool.tile([128, wm_max], dt, tag="wm")
                        nc.scalar.dma_start(out=t[:, :hi - lo], in_=wm[:, lo:hi])
                        for gi in range(g0, g1):
                            wm_t[gi] = (t, lo)

                flush_from = 0
                for gi, (g, capw, out_off, mms) in enumerate(gens):
                    ps = pspool.tile([128, GEN_COLS], mybir.dt.float32, tag="ps")
                    n = len(mms)
                    t, lo = wm_t[gi]
                    for mi, (bi, cl, ch, wc) in enumerate(mms):
                        nc.tensor.matmul(
                            ps[:, cl:ch], map_t[:, bi * C:(bi + 1) * C],
                            t[:, wc - lo:wc - lo + ch - cl],
                            start=(mi == 0), stop=(mi == n - 1))
                    cw = min(-(-capw // 8) * 8, GEN_COLS)
                    nc.vector.tensor_copy(stage[:, out_off:out_off + cw],
                                          ps[:, :cw])
                    if gi % 2 == 1 or gi == len(gens) - 1:
                        hi = out_off + cw
                        nc.sync.dma_start(out=o[:, flush_from:hi],
                                          in_=stage[:, flush_from:hi])
                        flush_from = hi

    _split_drains(nc, mybir, bass_rust)
    nc.finalize()
    return nc


_prog_cache = {}


def _get_program(meta, rep=1):
    key = (meta["gens"], meta["nblk"], meta["nslots"], meta["WCm"], rep)
    if key not in _prog_cache:
        _prog_cache[key] = _build_program(meta, rep=rep)
    return _prog_cache[key]


def _run(nc, in_maps):
    from concourse.bass_utils import run_bass_kernel_spmd
    last_err = None
    for _attempt in range(3):
        try:
            res = run_bass_kernel_spmd(nc, in_maps, list(range(NCORES)))
            return res.results
        except Exception as e:  # transient device wedge -> retry
            last_err = e
            time.sleep(2.0)
    raise last_err


# ----------------------------------------------------------------------------
# public entry
# ----------------------------------------------------------------------------

def kernel(data, rois, offset):
    data = np.asarray(data, f32)
    rois = np.asarray(rois, f32)
    offset = np.asarray(offset, f32)
    N = rois.shape[0]

    plan = _plan(rois, offset)
    if len(plan["meta"]["gens"]) == 0:
        return np.zeros((N, C, POOLED, POOLED), f32)
    in_maps = _build_inputs(plan, data)
    nc = _get_program(plan["meta"])
    results = _run(nc, in_maps)
    outs = [results[ci]["o"].astype(f32) for ci in range(NCORES)]
    return _gather(plan, outs)
